# revision 3
# baseline (speedup 1.0000x reference)
"""Trainium2 Bass kernel: 3x3 stride-1 pad-1 Conv2D, NCHW.

Problem: x (32,128,56,56) f32, weight (256,128,3,3) OIHW, bias (256,)
-> out (32,256,56,56) f32.

Strategy: data-parallel over batch N across 8 NeuronCores (4 images per
core), weights/bias replicated. Per core: implicit GEMM in fp8 e4m3
using the PE's DoubleRow perf mode (2 independent K=128 GEMMs per
instruction at 0.5 cycles/row = 157 TF/s). Precision is recovered with
a hi/lo residual scheme, all quantization done host-side:

  u = 32*x ~ Uhi + Ulo/16     (Uhi = Q(u), Ulo = Q(16*(u-Uhi)))
  v = 4096*w,  A = Q(v), B = Q(v/16), C = Q(v - A)

Per tap k the "main" DoubleRow pair computes A*Uhi + B*Ulo
(= v*u with only second-order error), and 4 "w-corr" pairs add C*Uhi
for 8 of 9 taps (fixes A's quantization error; only tap 2 left
uncorrected). 13 matmuls of 224 cycles per (row-tile, cout-chunk) vs
18x448 for fp16: ~2.8x fewer PE cycles, max rel err ~1.1e-2 (< 2e-2).

The SBUF x tile holds 4 copies of each padded row-chunk
[Uhi, Uhi+1, Ulo, Uhi+WP] so every pair's rhs is a (step-)sliced AP:
 - main pair:    copies (0,2): x[:, 0:3:2, kh:kh+R, kw:kw+56]
 - C pair kw 0/1: copies (0,1): x[:, 0:2, kh:kh+R, 0:56]
 - C pair (5,8): copies (0,3): x[:, 0:4:3, 1:1+R, 2:58]
PSUM (f32, exact) is dequantized by 2^-17, bias added, stored as fp16
(|y| ~ 2), upcast to f32 on host.
"""

import numpy as np
import ml_dtypes

import concourse.bass as bass
import concourse.mybir as mybir
import concourse.tile as tile
from concourse import bacc
from concourse.bass_utils import run_bass_kernel_spmd

N_CORES = 8
N_FULL = 32
N_PER_CORE = N_FULL // N_CORES  # 4
CIN = 128
COUT = 256
H = W = 56
HP = WP = 58  # padded spatial
HPWP_PAD = HP * WP + WP  # flat per-(n,ci) length incl. +WP shift slack
R = 8  # output rows per matmul tile
NT = H // R  # 7 row-tiles per image
NFREE = R * W  # 448 (<= 512 PSUM-bank limit per matmul)
NPAIR = 13  # 9 main pairs + 4 w-corr pairs
F32 = mybir.dt.float32
F16 = mybir.dt.float16
F8 = mybir.dt.float8e4
E4 = ml_dtypes.float8_e4m3fn
DR = mybir.MatmulPerfMode.DoubleRow

SX = 32.0  # x pre-scale (fp8 operand = 32*x)
SW = 4096.0  # w pre-scale
DEQ = 1.0 / (SX * SW)  # 2^-17, exact in f32

# C-pair tap layout: pair j covers taps (a_j, b_j); slot1's window comes
# from the shifted copy (copy1 = +1 elem for kw pairs, copy3 = +WP for (5,8)).
C_PAIRS = [(0, 1), (3, 4), (5, 8), (6, 7)]

# Module-level knobs for the dev harness (test.py). The grading harness
# just calls kernel(**inputs) and gets the default (no-trace) path.
TRACE = False
LAST_RESULT = None

_prog = None


def _build_program():
    nc = bacc.Bacc("TRN2", target_bir_lowering=False, debug=False)
    xh_d = nc.declare_dram_parameter("xh", [N_PER_CORE, CIN, HPWP_PAD], F8, isOutput=False)
    xl_d = nc.declare_dram_parameter("xl", [N_PER_CORE, CIN, HPWP_PAD], F8, isOutput=False)
    w_d = nc.declare_dram_parameter("wt", [CIN, 2 * NPAIR * 2 * 128], F8, isOutput=False)
    b_d = nc.declare_dram_parameter("bias", [COUT], F32, isOutput=False)
    out_d = nc.declare_dram_parameter(
        "out", [N_PER_CORE, 2, 128, H * W], F16, isOutput=True
    )

    CH = (R + 2) * WP  # one chunk: R output rows + 2 halo rows of padded input
    WB = NPAIR * 2 * 128  # weight bytes/partition per cout-chunk

    with tile.TileContext(nc) as tc:
        with (
            tc.tile_pool(name="const", bufs=1) as const_pool,
            tc.tile_pool(name="xin", bufs=4) as x_pool,
            tc.tile_pool(name="outp", bufs=4) as out_pool,
            tc.tile_pool(name="psum", bufs=6, space="PSUM") as psum_pool,
        ):
            # Weights on the sync engine, c=0 half first (the only half the
            # first 13 matmuls need); x chunks dispatch in parallel on the
            # scalar engine (the second HWDGE-capable engine).
            w_sbs = []
            for c in range(2):
                w_c = const_pool.tile([CIN, WB], F8, tag=f"w{c}")
                nc.sync.dma_start(out=w_c[:], in_=w_d[:, c * WB : (c + 1) * WB])
                w_sbs.append(w_c[:].rearrange("p (j two m) -> p j two m", j=NPAIR, two=2))
            bias_sb = const_pool.tile([128, 2], F32)

            # Warmup: dummy matmuls on a memset scratch tile fill the PE during
            # the initial DMA wait, so HAM un-throttles (needs ~3.4us of
            # sustained PE activity) before the first real matmul.
            scratch = const_pool.tile([128, 2 * NFREE], F8)
            nc.vector.memset(scratch[:], 0.0)
            scr_w = const_pool.tile([128, 2 * 128], F8)
            nc.vector.memset(scr_w[:], 0.0)
            scr_rhs = scratch[:].rearrange("p (two n) -> p two n", two=2)
            scr_lhs = scr_w[:].rearrange("p (two m) -> p two m", two=2)
            warm_ps = psum_pool.tile([128, NFREE], F32, tag="warm", bufs=1)
            NWARM = 20
            for wi in range(NWARM):
                nc.tensor.matmul(
                    warm_ps[:], lhsT=scr_lhs, rhs=scr_rhs,
                    start=(wi == 0), stop=(wi == NWARM - 1),
                    perf_mode=DR, skip_group_check=True,
                )

            # Per-image, per-row-block input chunks (overlapping halo rows) so
            # the first matmuls only wait on small DMAs, not whole images.
            x_tiles = {}

            def load_chunk(i, r):
                x_c = x_pool.tile([CIN, 4 * CH], F8)
                base = r * R * WP
                # copies: 0 = Uhi, 1 = Uhi+1, 2 = Ulo, 3 = Uhi+WP
                for ci, (src, off) in enumerate(
                    [(xh_d, 0), (xh_d, 1), (xl_d, 0), (xh_d, WP)]
                ):
                    nc.scalar.dma_start(
                        out=x_c[:, ci * CH : (ci + 1) * CH],
                        in_=src[i][:, base + off : base + off + CH],
                    )
                x_tiles[(i, r)] = x_c

            def compute_tile(i, c, r, row0=0, nrows=R, store_eng=None):
                x_v = x_tiles[(i, r)][:].rearrange(
                    "p (cp h w) -> p cp h w", cp=4, w=WP
                )
                nf = nrows * W
                psum_t = psum_pool.tile([128, NFREE], F32)
                out_ps = psum_t[:, :nf]
                n_mm = NPAIR
                for j in range(n_mm):
                    if j < 9:  # main pair for tap j: (A*Uhi + B*Ulo)
                        kh, kw = divmod(j, 3)
                        rhs = x_v[:, 0:3:2, row0 + kh : row0 + kh + nrows, kw : kw + W]
                    else:  # w-corr pair: (C_a*Uhi_a + C_b*Uhi_b)
                        a, b = C_PAIRS[j - 9]
                        kh, kw = divmod(a, 3)
                        if (a, b) == (5, 8):  # shift +WP copy
                            rhs = x_v[:, 0:4:3, row0 + kh : row0 + kh + nrows, kw : kw + W]
                        else:  # shift +1 copy
                            rhs = x_v[:, 0:2, row0 + kh : row0 + kh + nrows, kw : kw + W]
                    nc.tensor.matmul(
                        out_ps, lhsT=w_sbs[c][:, j], rhs=rhs,
                        start=(j == 0), stop=(j == n_mm - 1), perf_mode=DR,
                    )
                out_t = out_pool.tile([128, NFREE], F16)
                nc.vector.tensor_scalar(
                    out=out_t[:, :nf], in0=psum_t[:, :nf],
                    scalar1=DEQ, scalar2=bias_sb[:, c : c + 1],
                    op0=mybir.AluOpType.mult, op1=mybir.AluOpType.add,
                )
                lo = r * NFREE + row0 * W
                (store_eng or nc.sync).dma_start(
                    out=out_d[i, c][:, lo : lo + nf], in_=out_t[:, :nf]
                )

            # Emission order = DMA queue order: first two chunks land before
            # compute starts; each chunk is consumed by both co-chunks, then
            # its pool slot recycles.
            load_chunk(0, 0)
            load_chunk(0, 1)
            # Bias is tiny but DMAs as 256 4-byte descriptors; emit it after
            # the critical-path loads (first needed at the first copy-out).
            for c in range(2):
                nc.scalar.dma_start(
                    out=bias_sb[:, c : c + 1],
                    in_=b_d[c * 128 : (c + 1) * 128].rearrange("(p one) -> p one", one=1),
                )
            for i in range(N_PER_CORE):
                for r in range(NT):
                    nxt = (i, r + 2) if r + 2 < NT else (i + 1, (r + 2) % NT)
                    if nxt[0] < N_PER_CORE and nxt not in x_tiles:
                        load_chunk(*nxt)
                    last = i == N_PER_CORE - 1 and r == NT - 1
                    compute_tile(i, 0, r)
                    if last:
                        # Shorten the tail: the final copy-out + store chain
                        # handles 4 rows instead of 8.
                        compute_tile(i, 1, r, row0=0, nrows=4)
                        compute_tile(i, 1, r, row0=4, nrows=4)
                    else:
                        compute_tile(i, 1, r)
                    del x_tiles[(i, r)]
    nc.compile()
    return nc


def _quantize_inputs(x, weight):
    """Host-side fp8 prep. Returns (xh, xl, wt, shift) where shift is the
    extra power-of-2 the host must multiply back into the output (0 for
    data in the expected range)."""
    ax = float(np.abs(x).max())
    aw = float(np.abs(weight).max())
    # keep quantized magnitudes <= 224 (safe for e4m3 vs e4m3fn top codes)
    jx = max(0, int(np.ceil(np.log2(ax * SX / 224.0)))) if ax > 0 else 0
    jw = max(0, int(np.ceil(np.log2(aw * SW / 224.0)))) if aw > 0 else 0
    sx = SX / (1 << jx)
    sw = SW / (1 << jw)

    u = x * np.float32(sx)
    uhi = u.astype(E4)
    ulo = ((u - uhi.astype(np.float32)) * np.float32(16.0)).astype(E4)

    v = weight.astype(np.float64) * sw
    A = v.astype(np.float32).astype(E4)
    B = (v / 16.0).astype(np.float32).astype(E4)
    C = (v - A.astype(np.float64)).astype(np.float32).astype(E4)
    return uhi, ulo, A, B, C, jx + jw


def kernel(x: np.ndarray, weight: np.ndarray, bias: np.ndarray) -> np.ndarray:
    global _prog, LAST_RESULT
    x = np.ascontiguousarray(x, dtype=np.float32)
    weight = np.ascontiguousarray(weight, dtype=np.float32)
    bias = np.ascontiguousarray(bias, dtype=np.float32)

    uhi, ulo, A, B, C, shift = _quantize_inputs(x, weight)

    # Pad spatial dims into flat [N, CIN, HPWP_PAD] fp8 buffers.
    xh = np.zeros((N_FULL, CIN, HPWP_PAD), dtype=E4)
    xl = np.zeros((N_FULL, CIN, HPWP_PAD), dtype=E4)
    xh3 = xh[:, :, : HP * WP].reshape(N_FULL, CIN, HP, WP)
    xl3 = xl[:, :, : HP * WP].reshape(N_FULL, CIN, HP, WP)
    xh3[:, :, 1:-1, 1:-1] = uhi
    xl3[:, :, 1:-1, 1:-1] = ulo

    # Weight layout wt[ci, c, j, slot, co]: j<9 -> (A_j, B_j); j=9..12 ->
    # C-pairs per C_PAIRS. A/B/C are [cout, cin, 3, 3] in OIHW.
    def okc(m, c, k):  # [co=128, ci] slice for cout-chunk c, tap k
        kh, kw = divmod(k, 3)
        return m[c * 128 : (c + 1) * 128, :, kh, kw]

    wt = np.zeros((CIN, 2, NPAIR, 2, 128), dtype=E4)
    for c in range(2):
        for k in range(9):
            wt[:, c, k, 0, :] = okc(A, c, k).T
            wt[:, c, k, 1, :] = okc(B, c, k).T
        for j, (a, b) in enumerate(C_PAIRS):
            wt[:, c, 9 + j, 0, :] = okc(C, c, a).T
            wt[:, c, 9 + j, 1, :] = okc(C, c, b).T
    wt = np.ascontiguousarray(wt.reshape(CIN, 2 * NPAIR * 2 * 128))

    if _prog is None:
        _prog = _build_program()

    bias_dev = bias * np.float32(2.0**-shift) if shift else bias
    in_maps = [
        {
            "xh": np.ascontiguousarray(xh[i * N_PER_CORE : (i + 1) * N_PER_CORE]),
            "xl": np.ascontiguousarray(xl[i * N_PER_CORE : (i + 1) * N_PER_CORE]),
            "wt": wt,
            "bias": bias_dev,
        }
        for i in range(N_CORES)
    ]
    res = run_bass_kernel_spmd(_prog, in_maps, list(range(N_CORES)), trace=TRACE)
    LAST_RESULT = res
    out = np.concatenate([r["out"] for r in res.results], axis=0)
    out = out.astype(np.float32)
    if shift:
        out *= np.float32(2.0**shift)
    return out.reshape(N_FULL, COUT, H, W)


# revision 4
# speedup vs baseline: 1.4036x; 1.4036x over previous
"""Trainium2 Bass kernel: 3x3 stride-1 pad-1 Conv2D, NCHW.

Problem: x (32,128,56,56) f32, weight (256,128,3,3) OIHW, bias (256,)
-> out (32,256,56,56) f32.

Strategy: data-parallel over batch N across 8 NeuronCores (4 images per
core), weights/bias replicated. Per core: implicit GEMM — C_in=128 is
exactly the SBUF partition dim; for each of the 9 filter taps we issue a
128x128 (ci x co-chunk) matmul against a shifted window of the
host-padded image, accumulating all 9 taps into one PSUM bank. fp16
matmul (1 cycle/row) gives ~4x over plain fp32 at ~2.7e-4 rel err.
"""

import numpy as np

import concourse.bass as bass
import concourse.mybir as mybir
import concourse.tile as tile
from concourse import bacc
from concourse.bass_utils import run_bass_kernel_spmd

N_CORES = 8
N_FULL = 32
N_PER_CORE = N_FULL // N_CORES  # 4
CIN = 128
COUT = 256
H = W = 56
HP = WP = 58  # padded spatial
R = 8  # output rows per matmul tile
NT = H // R  # 7 row-tiles per image
NFREE = R * W  # 448 (<= 512 PSUM-bank limit per matmul)
F32 = mybir.dt.float32
F32R = mybir.dt.float32r
F16 = mybir.dt.float16

# Module-level knobs for the dev harness (test.py). The grading harness
# just calls kernel(**inputs) and gets the default (no-trace) path.
TRACE = False
LAST_RESULT = None

_prog = None


def _build_program():
    nc = bacc.Bacc("TRN2", target_bir_lowering=False, debug=False)
    x_d = nc.declare_dram_parameter("x", [N_PER_CORE, CIN, HP * WP], F16, isOutput=False)
    w_d = nc.declare_dram_parameter("wt", [CIN, 9 * COUT], F16, isOutput=False)
    b_d = nc.declare_dram_parameter("bias", [COUT], F32, isOutput=False)
    out_d = nc.declare_dram_parameter(
        "out", [N_PER_CORE, 2, 128, H * W], F32, isOutput=True
    )

    CH = (R + 2) * WP  # one chunk: R output rows + 2 halo rows of padded input

    with tile.TileContext(nc) as tc:
        with (
            tc.tile_pool(name="const", bufs=1) as const_pool,
            tc.tile_pool(name="xin", bufs=4) as x_pool,
            tc.tile_pool(name="outp", bufs=4) as out_pool,
            tc.tile_pool(name="psum", bufs=6, space="PSUM") as psum_pool,
        ):
            # Weights on the sync engine, c=0 half first (the only half the
            # first 9 matmuls need); x chunks dispatch in parallel on the
            # scalar engine (the second HWDGE-capable engine).
            w_sbs = []
            for c in range(2):
                w_c = const_pool.tile([CIN, 9 * 128], F16, tag=f"w{c}")
                nc.sync.dma_start(
                    out=w_c[:], in_=w_d[:, c * 9 * 128 : (c + 1) * 9 * 128]
                )
                w_sbs.append(w_c)
            bias_sb = const_pool.tile([128, 2], F32)

            # Warmup: dummy matmuls on a memset scratch tile fill the PE during
            # the initial DMA wait, so HAM un-throttles (needs ~3.4us of
            # sustained PE activity) before the first real matmul.
            scratch = const_pool.tile([128, NFREE], F16)
            nc.vector.memset(scratch[:], 0.0)
            warm_ps = psum_pool.tile([128, NFREE], F32, tag="warm", bufs=1)
            NWARM = 10
            for wi in range(NWARM):
                nc.tensor.matmul(
                    warm_ps[:], lhsT=scratch[:, :128], rhs=scratch[:],
                    start=(wi == 0), stop=(wi == NWARM - 1), skip_group_check=True,
                )

            # Per-image, per-row-block input chunks (overlapping halo rows) so
            # the first matmuls only wait on a ~300KB DMA, not whole images.
            x_view = x_d[:].rearrange("n p (h w) -> n p h w", w=WP)
            x_tiles = {}

            def load_chunk(i, r):
                x_c = x_pool.tile([CIN, CH], F16)
                nc.scalar.dma_start(
                    out=x_c[:],
                    in_=x_view[i][:, r * R : r * R + R + 2, :],
                )
                x_tiles[(i, r)] = x_c

            def compute_tile(i, c, r, row0=0, nrows=R, store_eng=None):
                x_img = x_tiles[(i, r)][:].rearrange("p (h w) -> p h w", w=WP)
                nf = nrows * W
                psum_t = psum_pool.tile([128, NFREE], F32)
                psum_v = psum_t[:, :nf].rearrange("p (r w) -> p r w", w=W)
                for k in range(9):
                    kh, kw = divmod(k, 3)
                    rhs = x_img[:, row0 + kh : row0 + kh + nrows, kw : kw + W]
                    lhsT = w_sbs[c][:, k * 128 : (k + 1) * 128]
                    nc.tensor.matmul(
                        psum_v, lhsT=lhsT, rhs=rhs, start=(k == 0), stop=(k == 8)
                    )
                out_t = out_pool.tile([128, NFREE], F32)
                nc.vector.tensor_scalar_add(
                    out_t[:, :nf], psum_t[:, :nf], bias_sb[:, c : c + 1]
                )
                lo = r * NFREE + row0 * W
                (store_eng or nc.sync).dma_start(
                    out=out_d[i, c][:, lo : lo + nf], in_=out_t[:, :nf]
                )

            # Emission order = DMA queue order: first two chunks land before
            # compute starts; each chunk is consumed by both co-chunks, then
            # its pool slot recycles.
            load_chunk(0, 0)
            load_chunk(0, 1)
            # Bias is tiny but DMAs as 256 4-byte descriptors; emit it after
            # the critical-path loads (first needed at the first copy-out).
            for c in range(2):
                nc.scalar.dma_start(
                    out=bias_sb[:, c : c + 1],
                    in_=b_d[c * 128 : (c + 1) * 128].rearrange("(p one) -> p one", one=1),
                )
            for i in range(N_PER_CORE):
                for r in range(NT):
                    nxt = (i, r + 2) if r + 2 < NT else (i + 1, (r + 2) % NT)
                    if nxt[0] < N_PER_CORE and nxt not in x_tiles:
                        load_chunk(*nxt)
                    last = i == N_PER_CORE - 1 and r == NT - 1
                    compute_tile(i, 0, r)
                    if last:
                        # Shorten the tail: the final copy-out + store chain
                        # handles 4 rows instead of 8.
                        compute_tile(i, 1, r, row0=0, nrows=4)
                        compute_tile(i, 1, r, row0=4, nrows=4)
                    else:
                        compute_tile(i, 1, r)
                    del x_tiles[(i, r)]
    nc.compile()
    return nc


def kernel(x: np.ndarray, weight: np.ndarray, bias: np.ndarray) -> np.ndarray:
    global _prog, LAST_RESULT
    x = np.ascontiguousarray(x, dtype=np.float32)
    weight = np.ascontiguousarray(weight, dtype=np.float32)
    bias = np.ascontiguousarray(bias, dtype=np.float32)

    # Host-side prep: pad spatial dims, shard batch, pre-transpose weights.
    x_pad = np.zeros((N_FULL, CIN, HP, WP), dtype=np.float16)
    x_pad[:, :, 1:-1, 1:-1] = x
    x_pad = x_pad.reshape(N_FULL, CIN, HP * WP)

    # wt[ci, (c*9 + k)*128 + co2] = weight[c*128 + co2, ci, kh, kw], k = kh*3+kw
    # (c-major so the c=0 half is one contiguous DMA)
    wt = np.ascontiguousarray(
        weight.reshape(2, 128, CIN, 9).transpose(2, 0, 3, 1).reshape(CIN, 9 * COUT)
    ).astype(np.float16)

    if _prog is None:
        _prog = _build_program()

    in_maps = [
        {
            "x": np.ascontiguousarray(x_pad[i * N_PER_CORE : (i + 1) * N_PER_CORE]),
            "wt": wt,
            "bias": bias,
        }
        for i in range(N_CORES)
    ]
    res = run_bass_kernel_spmd(_prog, in_maps, list(range(N_CORES)), trace=TRACE)
    LAST_RESULT = res
    out = np.concatenate([r["out"] for r in res.results], axis=0)
    return out.reshape(N_FULL, COUT, H, W)



# revision 7
# speedup vs baseline: 1.4053x; 1.0013x over previous
"""Trainium2 Bass kernel: 3x3 stride-1 pad-1 Conv2D, NCHW.

Problem: x (32,128,56,56) f32, weight (256,128,3,3) OIHW, bias (256,)
-> out (32,256,56,56) f32.

Strategy: data-parallel over batch N across 8 NeuronCores (4 images per
core), weights/bias replicated. Per core: implicit GEMM — C_in=128 is
exactly the SBUF partition dim; for each of the 9 filter taps we issue a
128x128 (ci x co-chunk) matmul against a shifted window of the
host-padded image, accumulating all 9 taps into one PSUM bank. fp16
matmul (1 cycle/row) gives ~4x over plain fp32 at ~2.7e-4 rel err.
"""

import numpy as np

import concourse.bass as bass
import concourse.mybir as mybir
import concourse.tile as tile
from concourse import bacc
from concourse.bass_utils import run_bass_kernel_spmd

N_CORES = 8
N_FULL = 32
N_PER_CORE = N_FULL // N_CORES  # 4
CIN = 128
COUT = 256
H = W = 56
HP = WP = 58  # padded spatial
R = 8  # output rows per matmul tile
NT = H // R  # 7 row-tiles per image
NFREE = R * W  # 448 (<= 512 PSUM-bank limit per matmul)
F32 = mybir.dt.float32
F32R = mybir.dt.float32r
F16 = mybir.dt.float16

# Module-level knobs for the dev harness (test.py). The grading harness
# just calls kernel(**inputs) and gets the default (no-trace) path.
TRACE = False
LAST_RESULT = None

_prog = None


def _build_program():
    nc = bacc.Bacc("TRN2", target_bir_lowering=False, debug=False)
    x_d = nc.declare_dram_parameter("x", [N_PER_CORE, CIN, HP * WP], F16, isOutput=False)
    w_d = nc.declare_dram_parameter("wt", [CIN, 9 * COUT], F16, isOutput=False)
    b_d = nc.declare_dram_parameter("bias", [COUT], F32, isOutput=False)
    # fp16 output (|y| <~ 2, rel err 2^-11): halves store traffic + teardown
    # fence latency vs f32; host upcasts.
    out_d = nc.declare_dram_parameter(
        "out", [N_PER_CORE, 2, 128, H * W], F16, isOutput=True
    )

    CH = (R + 2) * WP  # one chunk: R output rows + 2 halo rows of padded input

    with tile.TileContext(nc) as tc:
        with (
            tc.tile_pool(name="const", bufs=1) as const_pool,
            tc.tile_pool(name="xin", bufs=4) as x_pool,
            tc.tile_pool(name="outp", bufs=4) as out_pool,
            tc.tile_pool(name="psum", bufs=6, space="PSUM") as psum_pool,
        ):
            # Weights on the sync engine, c=0 half first (the only half the
            # first 9 matmuls need); x chunks dispatch in parallel on the
            # scalar engine (the second HWDGE-capable engine).
            w_sbs = []
            for c in range(2):
                w_c = const_pool.tile([CIN, 9 * 128], F16, tag=f"w{c}")
                nc.sync.dma_start(
                    out=w_c[:], in_=w_d[:, c * 9 * 128 : (c + 1) * 9 * 128]
                )
                w_sbs.append(w_c)
            bias_sb = const_pool.tile([128, 2], F32)

            # Warmup: dummy matmuls on a memset scratch tile fill the PE during
            # the initial DMA wait, so HAM un-throttles (needs ~3.4us of
            # sustained PE activity) before the first real matmul.
            scratch = const_pool.tile([128, NFREE], F16)
            nc.vector.memset(scratch[:], 0.0)
            warm_ps = psum_pool.tile([128, NFREE], F32, tag="warm", bufs=1)
            NWARM = 10
            for wi in range(NWARM):
                nc.tensor.matmul(
                    warm_ps[:], lhsT=scratch[:, :128], rhs=scratch[:],
                    start=(wi == 0), stop=(wi == NWARM - 1), skip_group_check=True,
                )

            # Per-image, per-row-block input chunks (overlapping halo rows) so
            # the first matmuls only wait on a ~300KB DMA, not whole images.
            x_view = x_d[:].rearrange("n p (h w) -> n p h w", w=WP)
            x_tiles = {}

            def load_chunk(i, r):
                x_c = x_pool.tile([CIN, CH], F16)
                nc.scalar.dma_start(
                    out=x_c[:],
                    in_=x_view[i][:, r * R : r * R + R + 2, :],
                )
                x_tiles[(i, r)] = x_c

            def compute_tile(i, c, r, row0=0, nrows=R, store_eng=None):
                x_img = x_tiles[(i, r)][:].rearrange("p (h w) -> p h w", w=WP)
                nf = nrows * W
                psum_t = psum_pool.tile([128, NFREE], F32)
                psum_v = psum_t[:, :nf].rearrange("p (r w) -> p r w", w=W)
                for k in range(9):
                    kh, kw = divmod(k, 3)
                    rhs = x_img[:, row0 + kh : row0 + kh + nrows, kw : kw + W]
                    lhsT = w_sbs[c][:, k * 128 : (k + 1) * 128]
                    nc.tensor.matmul(
                        psum_v, lhsT=lhsT, rhs=rhs, start=(k == 0), stop=(k == 8)
                    )
                out_t = out_pool.tile([128, NFREE], F16)
                nc.vector.tensor_scalar_add(
                    out_t[:, :nf], psum_t[:, :nf], bias_sb[:, c : c + 1]
                )
                lo = r * NFREE + row0 * W
                # Alternate store queue: spreads posted writes across both
                # HWDGE queues so neither backs up at the end.
                eng = store_eng or (nc.sync if c == 0 else nc.scalar)
                eng.dma_start(
                    out=out_d[i, c][:, lo : lo + nf], in_=out_t[:, :nf]
                )

            # Emission order = DMA queue order: first two chunks land before
            # compute starts; each chunk is consumed by both co-chunks, then
            # its pool slot recycles.
            load_chunk(0, 0)
            load_chunk(0, 1)
            # Bias is tiny but DMAs as 256 4-byte descriptors; emit it after
            # the critical-path loads (first needed at the first copy-out).
            for c in range(2):
                nc.scalar.dma_start(
                    out=bias_sb[:, c : c + 1],
                    in_=b_d[c * 128 : (c + 1) * 128].rearrange("(p one) -> p one", one=1),
                )
            for i in range(N_PER_CORE):
                for r in range(NT):
                    nxt = (i, r + 2) if r + 2 < NT else (i + 1, (r + 2) % NT)
                    if nxt[0] < N_PER_CORE and nxt not in x_tiles:
                        load_chunk(*nxt)
                    last = i == N_PER_CORE - 1 and r == NT - 1
                    compute_tile(i, 0, r)
                    if last:
                        # Shorten the tail: the final copy-out + store chain
                        # handles 4 rows instead of 8.
                        compute_tile(i, 1, r, row0=0, nrows=4)
                        compute_tile(i, 1, r, row0=4, nrows=4)
                    else:
                        compute_tile(i, 1, r)
                    del x_tiles[(i, r)]
    nc.compile()
    return nc


def kernel(x: np.ndarray, weight: np.ndarray, bias: np.ndarray) -> np.ndarray:
    global _prog, LAST_RESULT
    x = np.ascontiguousarray(x, dtype=np.float32)
    weight = np.ascontiguousarray(weight, dtype=np.float32)
    bias = np.ascontiguousarray(bias, dtype=np.float32)

    # Host-side prep: pad spatial dims, shard batch, pre-transpose weights.
    x_pad = np.zeros((N_FULL, CIN, HP, WP), dtype=np.float16)
    x_pad[:, :, 1:-1, 1:-1] = x
    x_pad = x_pad.reshape(N_FULL, CIN, HP * WP)

    # wt[ci, (c*9 + k)*128 + co2] = weight[c*128 + co2, ci, kh, kw], k = kh*3+kw
    # (c-major so the c=0 half is one contiguous DMA)
    wt = np.ascontiguousarray(
        weight.reshape(2, 128, CIN, 9).transpose(2, 0, 3, 1).reshape(CIN, 9 * COUT)
    ).astype(np.float16)

    if _prog is None:
        _prog = _build_program()

    in_maps = [
        {
            "x": np.ascontiguousarray(x_pad[i * N_PER_CORE : (i + 1) * N_PER_CORE]),
            "wt": wt,
            "bias": bias,
        }
        for i in range(N_CORES)
    ]
    res = run_bass_kernel_spmd(_prog, in_maps, list(range(N_CORES)), trace=TRACE)
    LAST_RESULT = res
    out = np.concatenate([r["out"] for r in res.results], axis=0)
    return out.astype(np.float32).reshape(N_FULL, COUT, H, W)

